# revision 1
# baseline (speedup 1.0000x reference)
"""CTC mean loss on 8 trn2 NeuronCores (Bass/Tile).

Per example: linear-domain CTC forward DP with per-t normalizer
ghat = pb + den/C. Label-position recurrences run as tensor_tensor_scan
over time in the (data0 add state) mult data1 form:
    l_i[t] = (u_i[t] + l_i[t-1]) * qn_i[t],   u_i = b_{i-1}' + d_i*l_{i-1}'
    b_i[t] = (l_i[t-1] + b_i[t-1]) * qb[t]
(4 label chunks x 4 time segments wavefront over 128 partitions, with
per-chunk boundary rescaling, log-scale tracked, for fp32 range control.)

loss = -( log(l_fin + b_fin) + sum_t log ghat - sum_t log den - S )

The label gather is done on the host (numpy take_along_axis) and shipped
pre-arranged in wave-block layout (bf16), so the device never runs
indirect DMAs; softmax denominators come from ACT-exp accum_out over
[t, c] tiles plus one small PE transpose per segment.
"""
import numpy as np

# problem constants (fixed by the spec)
B, T, C, L = 256, 512, 128, 128
NCORE = 8
BLOC = B // NCORE          # 32 examples per core
NCH, JW = 4, 32            # label chunks x positions per chunk (i = 32q+j+1)
NSEG, TSEG = 4, 128        # time segments
NW = NSEG + NCH - 1        # 7 waves
BLANK = C - 1
LT = 38.0                  # log rescale target
TINY = 4e-18               # >= 2^-64 after sqrt (ACT Ln domain)

_PROG = {}


def _wave_ranges(w):
    qlo, qhi = max(0, w - (NSEG - 1)), min(NCH - 1, w)
    return qlo, qhi, 32 * qlo, 32 * (qhi + 1)


MAXEND = {0: 128, 32: 64, 64: 128, 96: 128}


def qsplit(a, b, shifted=False):
    """Decompose [a, b) into SBUF-legal partition ranges (quadrant rules);
    with shifted=True the range shifted down 32 must also be legal."""
    out = []
    while a < b:
        e = min(b, MAXEND[a])
        if shifted and a >= 32:
            e = min(e, MAXEND[a - 32] + 32)
        if e <= a:
            e = min(c for c in (32, 64, 96, 128) if c > a)
        out.append((a, e))
        a = e
    return out


def _patch_drain():
    """This container's walrus rejects TPB_CTRL drains with >1 sem wait;
    split the TileContext exit drain into one-wait-per-drain instructions."""
    import bass_rust
    import concourse.tile as tile_mod
    from concourse.tile import ScopedClock

    if getattr(tile_mod.TileContext, "_drain_split_patch", False):
        return

    def patched(self, tick_clock, wait_clock):
        drain_inst = self.nc.sync.drain()
        wait_clock.add_sem_waits(
            drain_inst.ins, ScopedClock({None: tick_clock.global_clock})
        )
        si = drain_inst.ins.sync_info
        waits = list(si.on_wait) if si is not None else []
        if len(waits) > 1:
            drain_inst.ins.sync_info = bass_rust.SyncInfo(
                on_wait=waits[:1], on_update=list(si.on_update)
            )
            for i in range(1, len(waits)):
                extra = self.nc.sync.drain()
                extra.ins.sync_info = bass_rust.SyncInfo(
                    on_wait=[waits[i]], on_update=[]
                )
        self.nc.all_engine_barrier()
        popped = self.nc._tile_sem_poison_stack.pop()
        assert popped is self._sem_poison
        self.nc.clear_and_free_semaphores(list(self.sems.allocated().values()))
        self.nc.all_engine_barrier()

    tile_mod.TileContext._drain_and_barrier = patched
    tile_mod.TileContext._drain_split_patch = True


def _split_waits(nc):
    """This container's walrus accepts at most ONE sem wait per instruction;
    hoist extra waits onto same-engine NoOps inserted just before."""
    import bass_rust

    cnt = 0
    for f in nc.m.functions:
        for bb in f.blocks:
            new = []
            changed = False
            for inst in bb.instructions:
                si = inst.sync_info
                waits = list(si.on_wait) if si is not None else []
                if len(waits) > 1:
                    changed = True
                    for wt in waits[:-1]:
                        cnt += 1
                        nop = bass_rust.InstNoOp(
                            name=f"I-wsplit-{cnt}", engine=inst.engine
                        )
                        nop.sync_info = bass_rust.SyncInfo(on_wait=[wt], on_update=[])
                        new.append(nop)
                    inst.sync_info = bass_rust.SyncInfo(
                        on_wait=[waits[-1]], on_update=list(si.on_update)
                    )
                new.append(inst)
            if changed:
                bb.instructions = new
    return cnt


def build_program(split_waits=True):
    import concourse.bass as bass
    import concourse.mybir as mybir
    from concourse.tile import TileContext
    from concourse.masks import make_identity

    _patch_drain()
    f32 = mybir.dt.float32
    bf16 = mybir.dt.bfloat16
    Alu = mybir.AluOpType
    Act = mybir.ActivationFunctionType
    Ax = mybir.AxisListType

    nc = bass.Bass()
    shiftm = nc.declare_dram_parameter("shiftm", [128, 256], f32, isOutput=False)
    zlw = nc.declare_dram_parameter("zlw", [512 * JW, TSEG], bf16, isOutput=False)
    ydN = nc.declare_dram_parameter("ydN", [BLOC * NSEG * TSEG, C], f32, isOutput=False)
    zbk = nc.declare_dram_parameter("zbk", [BLOC, NSEG * TSEG], f32, isOutput=False)
    dcol_in = nc.declare_dram_parameter("dcol", [128, JW], f32, isOutput=False)
    out_tl = nc.declare_dram_parameter("out_tl", [128, JW], bf16, isOutput=True)
    out_tb = nc.declare_dram_parameter("out_tb", [128, JW], bf16, isOutput=True)
    out_S = nc.declare_dram_parameter("out_S", [128, 1], f32, isOutput=True)
    out_logg = nc.declare_dram_parameter("out_logg", [BLOC, NSEG], f32, isOutput=True)
    out_logden = nc.declare_dram_parameter("out_logden", [BLOC, NSEG], f32, isOutput=True)

    with TileContext(nc) as tc:
        with (
            tc.tile_pool(name="pers", bufs=1) as pers,
            tc.tile_pool(name="psum", bufs=2, space="PSUM") as psum,
            tc.tile_pool(name="load", bufs=6) as loadp,
            tc.tile_pool(name="work", bufs=6) as workp,
            tc.tile_pool(name="ldp", bufs=4) as ldpool,
            tc.tile_pool(name="cols", bufs=2) as colp,
        ):
            # ---------------- persistent state ----------------
            CURL = [pers.tile([128, JW, TSEG + 1], bf16, name=f"curl{p}", tag=f"curl{p}") for p in range(2)]
            CURB = [pers.tile([128, JW, TSEG + 1], bf16, name=f"curb{p}", tag=f"curb{p}") for p in range(2)]
            QN = [pers.tile([128, JW, TSEG + 1], bf16, name=f"qn{p}", tag=f"qn{p}") for p in range(2)]
            UT = [pers.tile([128, JW, TSEG + 1], bf16, name=f"ut{p}", tag=f"ut{p}") for p in range(2)]
            PREVL = [pers.tile([128, TSEG], bf16, name=f"prevl{p}", tag=f"prevl{p}") for p in range(2)]
            PREVB = [pers.tile([128, TSEG], bf16, name=f"prevb{p}", tag=f"prevb{p}") for p in range(2)]
            B0 = [pers.tile([32, TSEG + 1], f32, name=f"b0{p}", tag=f"b0{p}") for p in range(2)]
            QB = [pers.tile([128, TSEG], bf16, name=f"qb{w}", tag=f"qb{w}") for w in range(NW)]
            RG = [pers.tile([128, TSEG], bf16, name=f"rg{w}", tag=f"rg{w}") for w in range(NW)]
            DCOL = pers.tile([128, JW], f32, name="dcol", tag="dcol")
            IDENT = pers.tile([128, 128], f32, name="ident", tag="ident")
            # partition-shift matrices for the PE (col m reads row m-32):
            # SHF: pure shift (rows 0:32 zero, for PREV); SHI: shift with
            # identity on rows 0:32 (for the S snapshot).
            SHF = pers.tile([128, 128], bf16, name="shf", tag="shf")
            SHI = pers.tile([128, 128], f32, name="shi", tag="shi")
            SDEN = pers.tile([128, NSEG * BLOC], f32, name="sden", tag="sden")
            Scol = pers.tile([128, 1], f32, name="scol", tag="scol")
            Sshift = pers.tile([128, 1], f32, name="sshift", tag="sshift")
            rcol = pers.tile([128, 1], f32, name="rcol", tag="rcol")
            LOGG = pers.tile([BLOC, NSEG], f32, name="logg", tag="logg")
            LOGDEN = pers.tile([BLOC, NSEG], f32, name="logden", tag="logden")
            ZBK = pers.tile([BLOC, NSEG * TSEG], f32, name="zbkt", tag="zbkt")

            # ---------------- init ----------------
            with nc.named_scope("init"):
                nc.sync.dma_start(out=DCOL[:], in_=dcol_in[:])
                nc.sync.dma_start(out=ZBK[:], in_=zbk[:])
                nc.sync.dma_start(out=SHI[:], in_=shiftm[:, 0:128])
                shf_f = workp.tile([128, 128], f32, name="shf_f", tag="shf_f")
                nc.sync.dma_start(out=shf_f[:], in_=shiftm[:, 128:256])
                nc.vector.tensor_copy(out=SHF[:], in_=shf_f[:])
                for p in range(2):
                    nc.gpsimd.memset(CURL[p][:], 0.0)
                    nc.gpsimd.memset(CURB[p][:], 0.0)
                    nc.gpsimd.memset(QN[p][:], 0.0)
                    nc.gpsimd.memset(QN[p][:, :, 0:1], 1.0)
                    nc.gpsimd.memset(PREVL[p][:], 0.0)
                    nc.gpsimd.memset(PREVB[p][:], 0.0)
                    nc.gpsimd.memset(B0[p][:], 0.0)
                nc.gpsimd.memset(B0[1][:, TSEG:TSEG + 1], 1.0)
                for w in range(NW):
                    nc.gpsimd.memset(QB[w][:], 0.0)
                    nc.gpsimd.memset(RG[w][:], 0.0)
                make_identity(nc, IDENT[:])
                nc.gpsimd.memset(Scol[:], 0.0)
                nc.gpsimd.memset(Sshift[:], 0.0)
                nc.gpsimd.memset(rcol[:], 1.0)

            # ---------------- stats pre-phase, per segment k ----------------
            # den[b, t] via ACT exp accum over the class axis ([t, c] tiles
            # from ydN), transposed to [b, t] by one PE transpose per k.
            ydN_v = ydN.rearrange("(b k t) c -> b k t c", b=BLOC, k=NSEG)
            for k in range(NSEG):
                with nc.named_scope(f"stats{k}"):
                    pbt = workp.tile([BLOC, TSEG], f32, name="pbt", tag="pbt")
                    gk = workp.tile([BLOC, TSEG], f32, name="gk", tag="gk")
                    rgk = workp.tile([BLOC, TSEG], f32, name="rgk", tag="rgk")
                    qbk = workp.tile([BLOC, TSEG], f32, name="qbk", tag="qbk")
                    rgkh = workp.tile([BLOC, TSEG], bf16, name="rgkh", tag="rgkh")
                    qbkh = workp.tile([BLOC, TSEG], bf16, name="qbkh", tag="qbkh")
                    scr = workp.tile([BLOC, TSEG], f32, name="scr", tag="scr")
                    denp = psum.tile([BLOC, TSEG], f32, name="denp", tag="denp")
                    nc.scalar.activation(out=pbt[:], in_=ZBK[:, k * TSEG:(k + 1) * TSEG],
                                         func=Act.Exp)
                    for b in range(BLOC):
                        zt = loadp.tile([TSEG, C], f32, name="zt", tag="zt")
                        pt = loadp.tile([TSEG, C], f32, name="pt", tag="pt")
                        nc.sync.dma_start(out=zt[:], in_=ydN_v[b, k, :, :])
                        nc.scalar.activation(out=pt[:], in_=zt[:], func=Act.Exp,
                                             accum_out=SDEN[:, 32 * k + b:32 * k + b + 1])
                    nc.tensor.transpose(out=denp[:], in_=SDEN[:, 32 * k:32 * k + 32],
                                        identity=IDENT[:])
                    # g = den/C + pb ; rg = 1/g ; qb = pb*rg
                    nc.vector.scalar_tensor_tensor(
                        out=gk[:], in0=denp[:], scalar=1.0 / C, in1=pbt[:],
                        op0=Alu.mult, op1=Alu.add)
                    nc.vector.reciprocal(out=rgk[:], in_=gk[:])
                    nc.vector.tensor_tensor(out=qbk[:], in0=pbt[:], in1=rgk[:], op=Alu.mult)
                    nc.vector.tensor_copy(out=rgkh[:], in_=rgk[:])
                    nc.vector.tensor_copy(out=qbkh[:], in_=qbk[:])
                    nc.scalar.activation(out=scr[:], in_=gk[:], func=Act.Ln,
                                         accum_out=LOGG[:, k:k + 1])
                    nc.scalar.activation(out=scr[:], in_=denp[:], func=Act.Ln,
                                         accum_out=LOGDEN[:, k:k + 1])
                    for q in range(NCH):
                        w = q + k
                        rows = slice(32 * q, 32 * q + 32)
                        nc.sync.dma_start(out=QB[w][rows, :], in_=qbkh[:])
                        nc.sync.dma_start(out=RG[w][rows, :], in_=rgkh[:])
            nc.sync.dma_start(out=out_logg[:], in_=LOGG[:])
            nc.sync.dma_start(out=out_logden[:], in_=LOGDEN[:])

            # ---------------- DP waves ----------------
            zlw_base = [0] * NW
            acc = 0
            for w in range(NW):
                zlw_base[w] = acc
                qlo, qhi, p0, p1 = _wave_ranges(w)
                acc += (qhi - qlo + 1) * 32 * JW

            for w in range(NW):
                P = w % 2
                qlo, qhi, p0, p1 = _wave_ranges(w)
                CL, CB = CURL[P], CURB[P]
                OL, OB = CURL[1 - P], CURB[1 - P]
                PL, PB = PREVL[P], PREVB[P]
                U = UT[P]
                B0c, B0o = B0[P], B0[1 - P]

                with nc.named_scope(f"emit{w}"):
                    # gathered logits for this wave -> QN slots 1..TSEG
                    # (gpsimd DMA ring: keeps these bulk loads off the sync
                    # ring so the small wstart DMAs don't queue behind them)
                    zlw_v = zlw.rearrange("(r j) t -> r j t", j=JW)
                    for a, b in qsplit(p0, p1):
                        r0 = zlw_base[w] // JW + (a - p0)
                        nc.sync.dma_start(
                            out=QN[P][a:b, :, 1:TSEG + 1],
                            in_=zlw_v[r0:r0 + (b - a), :, :],
                        )
                        nc.scalar.activation(out=QN[P][a:b, :, 1:TSEG + 1],
                                             in_=QN[P][a:b, :, 1:TSEG + 1], func=Act.Exp)
                        nc.gpsimd.tensor_tensor(
                            out=QN[P][a:b, :, 1:TSEG + 1],
                            in0=QN[P][a:b, :, 1:TSEG + 1],
                            in1=RG[w][a:b, None, :].to_broadcast([b - a, JW, TSEG]),
                            op=Alu.mult)

                with nc.named_scope(f"wstart{w}"):
                    if w >= 1:
                        r1 = 32 * (min(NCH - 1, w - 1) + 1)  # rescale rows [p0, r1)
                        pr0 = max(32, p0)
                        m_own = colp.tile([128, 1], f32, name="m_own", tag="m_own")
                        m_in = colp.tile([128, 1], f32, name="m_in", tag="m_in")
                        lg1 = colp.tile([128, 1], f32, name="lg1", tag="lg1")
                        lg2 = colp.tile([128, 1], f32, name="lg2", tag="lg2")
                        peak = colp.tile([128, 1], f32, name="peak", tag="peak")
                        snew = colp.tile([128, 1], f32, name="snew", tag="snew")
                        darg = colp.tile([128, 1], f32, name="darg", tag="darg")
                        # partition shifts on the idle PE (no DMA: keeps the
                        # inter-wave path off the DMA queues, which carry the
                        # large emit transfers)
                        psh = psum.tile([128, 1], f32, name="psh", tag="psh")
                        ppl = psum.tile([128, TSEG], f32, name="ppl", tag="ppl")
                        ppb = psum.tile([128, TSEG], f32, name="ppb", tag="ppb")
                        nc.tensor.matmul(out=psh[:], lhsT=SHI[:], rhs=Scol[:],
                                         start=True, stop=True)
                        nc.tensor.matmul(out=ppl[:], lhsT=SHF[:],
                                         rhs=OL[:, JW - 1, 0:TSEG], start=True, stop=True)
                        nc.tensor.matmul(out=ppb[:], lhsT=SHF[:],
                                         rhs=OB[:, JW - 1, 0:TSEG], start=True, stop=True)
                        nc.vector.tensor_copy(out=Sshift[:], in_=psh[:])
                        # all column math runs full-range [0:128]: inactive
                        # rows compute bounded garbage (their OL/PL inputs are
                        # 0/finite and exp args are clamped), and only the
                        # restricted S update below has lasting effect.
                        nc.vector.tensor_reduce(out=m_in[:], in_=ppl[:],
                                                axis=Ax.X, op=Alu.max)
                        nc.vector.tensor_reduce(out=lg2[:], in_=ppb[:],
                                                axis=Ax.X, op=Alu.max)
                        nc.vector.tensor_tensor(out=m_in[:], in0=m_in[:],
                                                in1=lg2[:], op=Alu.max)
                        if qlo == 0:
                            nc.vector.tensor_copy(out=m_in[0:32, :],
                                                  in_=B0o[:, TSEG:TSEG + 1])
                        nc.vector.tensor_reduce(out=m_own[:],
                                                in_=OL[:, :, TSEG:TSEG + 1],
                                                axis=Ax.XY, op=Alu.max)
                        nc.vector.tensor_reduce(out=lg1[:],
                                                in_=OB[:, :, TSEG:TSEG + 1],
                                                axis=Ax.XY, op=Alu.max)
                        nc.vector.tensor_tensor(out=m_own[:], in0=m_own[:],
                                                in1=lg1[:], op=Alu.max)
                        nc.vector.tensor_scalar_max(m_own[:], m_own[:], TINY)
                        nc.vector.tensor_scalar_max(m_in[:], m_in[:], TINY)
                        nc.vector.tensor_scalar_min(m_own[:], m_own[:], 1e38)
                        nc.vector.tensor_scalar_min(m_in[:], m_in[:], 1e38)
                        # bitcast log2/exp2 approximations on DVE: keeps the
                        # inter-wave chain off ACT (no table reloads, no
                        # cross-engine hops). ln(m) ~ (bits(m) - B)*KLN.
                        KLN = float(np.log(2.0) / (1 << 23))
                        FB = float(127 << 23)
                        iw1 = colp.tile([128, 1], mybir.dt.int32, name="iw1", tag="iw1")
                        iw2 = colp.tile([128, 1], mybir.dt.int32, name="iw2", tag="iw2")
                        nc.vector.tensor_copy(out=lg1[:], in_=m_own[:].bitcast(mybir.dt.int32))
                        nc.vector.tensor_copy(out=lg2[:], in_=m_in[:].bitcast(mybir.dt.int32))
                        nc.vector.tensor_scalar(out=lg1[:], in0=lg1[:],
                                                scalar1=KLN, scalar2=-FB * KLN,
                                                op0=Alu.mult, op1=Alu.add)
                        nc.vector.tensor_scalar(out=lg2[:], in0=lg2[:],
                                                scalar1=KLN, scalar2=-FB * KLN,
                                                op0=Alu.mult, op1=Alu.add)
                        nc.vector.tensor_tensor(out=lg1[:], in0=lg1[:],
                                                in1=Scol[:], op=Alu.subtract)
                        nc.vector.tensor_tensor(out=lg2[:], in0=lg2[:],
                                                in1=Sshift[:], op=Alu.subtract)
                        nc.vector.tensor_tensor(out=peak[:], in0=lg1[:],
                                                in1=lg2[:], op=Alu.max)
                        nc.vector.tensor_scalar(out=snew[:], in0=peak[:],
                                                scalar1=-1.0, scalar2=LT,
                                                op0=Alu.mult, op1=Alu.add)
                        # r = exp(S_new - S) via exp2 bitcast; S updated with
                        # the exact log of the applied factor (round-trip
                        # through the int bits); then corr = exp(S_upd - Sshift)
                        # from the UPDATED S so freshly-activated chunks get
                        # corr == 1 exactly.
                        nc.vector.tensor_tensor(out=darg[:], in0=snew[:],
                                                in1=Scol[:], op=Alu.subtract)
                        nc.vector.tensor_scalar_min(darg[:], darg[:], 80.0)
                        nc.vector.tensor_scalar_max(darg[:], darg[:], -85.0)
                        nc.vector.tensor_scalar(out=darg[:], in0=darg[:],
                                                scalar1=1.0 / KLN, scalar2=FB,
                                                op0=Alu.mult, op1=Alu.add)
                        nc.vector.tensor_copy(out=iw1[:], in_=darg[:])
                        nc.vector.tensor_copy(out=rcol[:], in_=iw1[:].bitcast(f32))
                        nc.vector.tensor_copy(out=darg[:], in_=iw1[:])
                        nc.vector.tensor_scalar(out=darg[:], in0=darg[:],
                                                scalar1=KLN, scalar2=-FB * KLN,
                                                op0=Alu.mult, op1=Alu.add)
                        for a, b in qsplit(p0, r1):
                            nc.vector.tensor_tensor(out=Scol[a:b, :], in0=Scol[a:b, :],
                                                    in1=darg[a:b, :], op=Alu.add)
                        nc.vector.tensor_tensor(out=peak[:], in0=Scol[:],
                                                in1=Sshift[:], op=Alu.subtract)
                        nc.vector.tensor_scalar_min(peak[:], peak[:], 80.0)
                        nc.vector.tensor_scalar_max(peak[:], peak[:], -85.0)
                        nc.vector.tensor_scalar(out=peak[:], in0=peak[:],
                                                scalar1=1.0 / KLN, scalar2=FB,
                                                op0=Alu.mult, op1=Alu.add)
                        nc.vector.tensor_copy(out=iw2[:], in_=peak[:])
                        nc.vector.tensor_copy(out=lg2[:], in_=iw2[:].bitcast(f32))
                        nc.vector.tensor_scalar_mul(PL[:], ppl[:], lg2[:])
                        nc.vector.tensor_scalar_mul(PB[:], ppb[:], lg2[:])
                    # batched boundary handoff: U[:, :, 0] and CB[:, :, 0]
                    nc.gpsimd.tensor_scalar_mul(U[:, :, 0:1], OL[:, :, TSEG:TSEG + 1],
                                                rcol[:])
                    nc.gpsimd.tensor_scalar_mul(CB[:, :, 0:1], OB[:, :, TSEG:TSEG + 1],
                                                rcol[:])
                    # b0 row (chunk 0 active in waves 0..NSEG-1)
                    if w <= NSEG - 1:
                        nc.vector.tensor_scalar_mul(B0c[:, 0:1], B0o[:, TSEG:TSEG + 1],
                                                    rcol[0:32, :])
                        nc.vector.tensor_tensor_scan(
                            out=B0c[:, 1:TSEG + 1], data0=QB[w][0:32, :],
                            data1=PREVL[P][0:32, :], initial=B0c[:, 0:1],
                            op0=Alu.mult, op1=Alu.add)
                        nc.gpsimd.tensor_copy(out=PB[0:32, :], in_=B0c[:, 0:TSEG])

                with nc.named_scope(f"jloop{w}"):
                    for j in range(JW):
                        if j == 0:
                            plp, pbp = PL[:, :], PB[:, :]
                        else:
                            plp = CL[:, j - 1, 0:TSEG]
                            pbp = CB[:, j - 1, 0:TSEG]
                        nc.vector.scalar_tensor_tensor(
                            out=U[:, j, 1:TSEG + 1], in0=plp,
                            scalar=DCOL[:, j:j + 1], in1=pbp,
                            op0=Alu.mult, op1=Alu.add)
                        nc.vector.tensor_tensor_scan(
                            out=CL[:, j, 0:TSEG + 1], data0=U[:, j, 0:TSEG + 1],
                            data1=QN[P][:, j, 0:TSEG + 1], initial=0.0,
                            op0=Alu.add, op1=Alu.mult)
                        nc.vector.tensor_tensor_scan(
                            out=CB[:, j, 1:TSEG + 1], data0=CL[:, j, 0:TSEG],
                            data1=QB[w][:, :], initial=CB[:, j, 0:1],
                            op0=Alu.add, op1=Alu.mult)

                if w >= NSEG - 1:  # chunk q = w - (NSEG-1) just finished segment 3
                    q = w - (NSEG - 1)
                    rows = slice(32 * q, 32 * q + 32)
                    nc.sync.dma_start(out=out_tl[rows, :], in_=CL[rows, :, TSEG:TSEG + 1])
                    nc.sync.dma_start(out=out_tb[rows, :], in_=CB[rows, :, TSEG:TSEG + 1])
                    nc.sync.dma_start(out=out_S[rows, :], in_=Scol[rows, :])
    if split_waits:
        _split_waits(nc)
    return nc


def host_prep(y_true, y_pred):
    """Build per-core input maps."""
    import ml_dtypes

    y_true = np.asarray(y_true).astype(np.int32)
    y_pred = np.asarray(y_pred).astype(np.float32)
    shiftm = np.zeros((128, 256), np.float32)
    for m in range(128):
        shiftm[m - 32 if m >= 32 else m, m] = 1.0          # SHI
        if m >= 32:
            shiftm[m - 32, 128 + m] = 1.0                  # SHF
    in_maps = []
    for core in range(NCORE):
        yt = y_true[core * BLOC:(core + 1) * BLOC]        # [32, 128]
        yp = y_pred[core * BLOC:(core + 1) * BLOC]        # [32, 512, 128]
        ydN = np.ascontiguousarray(yp).reshape(BLOC * NSEG * TSEG, C)
        zbk = np.ascontiguousarray(yp[:, :, BLANK])       # [32, 512]
        # gathered label logits, wave-block layout
        zg = np.take_along_axis(yp, yt.astype(np.int64)[:, None, :], axis=2)
        zgt = zg.transpose(0, 2, 1)                       # [32 b, 128 i(1..128), 512 t]
        zlw = np.zeros((512 * JW, TSEG), np.float32)
        base = 0
        for w in range(NW):
            qlo, qhi, p0, p1 = _wave_ranges(w)
            for q in range(qlo, qhi + 1):
                k = w - q
                blk = zgt[:, 32 * q:32 * q + JW, k * TSEG:(k + 1) * TSEG]  # [b, j, t]
                n = BLOC * JW
                zlw[base:base + n] = blk.reshape(n, TSEG)
                base += n
        zlw = zlw.astype(ml_dtypes.bfloat16)
        dcol = np.zeros((128, JW), np.float32)
        for q in range(NCH):
            for j in range(JW):
                i = 32 * q + j + 1
                if i >= 2:
                    dcol[32 * q:32 * q + 32, j] = (
                        yt[:, i - 1] != yt[:, i - 2]).astype(np.float32)
        in_maps.append({"zlw": zlw, "ydN": ydN, "zbk": zbk, "dcol": dcol,
                        "shiftm": shiftm})
    return in_maps


def host_finish(y_true, results):
    y_true = np.asarray(y_true)
    ll = (y_true != 0).sum(axis=1).astype(np.int64)        # [256]
    losses = np.zeros(B, np.float64)
    for core in range(NCORE):
        res = results[core]
        tl = np.asarray(res["out_tl"], dtype=np.float64)
        tb = np.asarray(res["out_tb"], dtype=np.float64)
        S, logg, logden = res["out_S"], res["out_logg"], res["out_logden"]
        for b in range(BLOC):
            gb = core * BLOC + b
            l = int(ll[gb])
            q, j = (l - 1) // 32, (l - 1) % 32
            p = 32 * q + b
            logP = np.log(tl[p, j] + tb[p, j])
            losses[gb] = -(logP + logg[b].sum() - logden[b].sum() - S[p, 0])
    return np.float32(losses.mean())


def _get_program():
    if "nc" not in _PROG:
        _PROG["nc"] = build_program()
    return _PROG["nc"]


def kernel(y_true: np.ndarray, y_pred: np.ndarray) -> np.ndarray:
    from concourse.bass_utils import run_bass_kernel_spmd

    nc = _get_program()
    in_maps = host_prep(y_true, y_pred)
    res = run_bass_kernel_spmd(nc, in_maps, core_ids=list(range(NCORE)))
    return host_finish(y_true, res.results)



# revision 2
# speedup vs baseline: 1.0900x; 1.0900x over previous
"""CTC mean loss on 8 trn2 NeuronCores (Bass/Tile) — restructured.

Per example: linear-domain CTC forward DP with per-t normalizer
g = pb + den/C. Label-position recurrences run as tensor_tensor_scan
over time; 4 label chunks x 4 time segments wavefront over 128
partitions, with per-wave boundary rescaling (log-scale tracked in Scol).

Layout (vs the previous version): CURL[., j, 0] holds the b-boundary so
the CB scan runs with an immediate 0.0 initial (AP initials cost ~86ns
extra per scan); CURL[., j, 1+t] = l_j[t].  Stats for segment k and the
emit (gathered-logit exp) for wave w+1 are issued behind jloop(w) and
run on ACT/DMA/gpsimd under the DVE wave grind.  The rescale chain uses
ACT exp instead of exact exp2 bitcast round-trips (S absorbs the tiny
log error), and outputs are staged into contiguous tiles and shipped
once at the end.
"""
import numpy as np

# problem constants (fixed by the spec)
B, T, C, L = 256, 512, 128, 128
NCORE = 8
BLOC = B // NCORE          # 32 examples per core
NCH, JW = 4, 32            # label chunks x positions per chunk (i = 32q+j+1)
NSEG, TSEG = 4, 128        # time segments
NW = NSEG + NCH - 1        # 7 waves
BLANK = C - 1
LT = 38.0                  # log rescale target
TINY = 4e-18
KLN = float(np.log(2.0) / (1 << 23))
FB = float(127 << 23)

_PROG = {}


def _wave_ranges(w):
    qlo, qhi = max(0, w - (NSEG - 1)), min(NCH - 1, w)
    return qlo, qhi, 32 * qlo, 32 * (qhi + 1)


MAXEND = {0: 128, 32: 64, 64: 128, 96: 128}


def qsplit(a, b):
    """Decompose [a, b) into SBUF-legal partition ranges (quadrant rules)."""
    out = []
    while a < b:
        e = min(b, MAXEND[a])
        if e <= a:
            e = min(c for c in (32, 64, 96, 128) if c > a)
        out.append((a, e))
        a = e
    return out


def _patch_drain():
    """This container's walrus rejects TPB_CTRL drains with >1 sem wait;
    split the TileContext exit drain into one-wait-per-drain instructions."""
    import bass_rust
    import concourse.tile as tile_mod
    from concourse.tile import ScopedClock

    if getattr(tile_mod.TileContext, "_drain_split_patch", False):
        return

    def patched(self, tick_clock, wait_clock):
        drain_inst = self.nc.sync.drain()
        wait_clock.add_sem_waits(
            drain_inst.ins, ScopedClock({None: tick_clock.global_clock})
        )
        si = drain_inst.ins.sync_info
        waits = list(si.on_wait) if si is not None else []
        if len(waits) > 1:
            drain_inst.ins.sync_info = bass_rust.SyncInfo(
                on_wait=waits[:1], on_update=list(si.on_update)
            )
            for i in range(1, len(waits)):
                extra = self.nc.sync.drain()
                extra.ins.sync_info = bass_rust.SyncInfo(
                    on_wait=[waits[i]], on_update=[]
                )
        self.nc.all_engine_barrier()
        popped = self.nc._tile_sem_poison_stack.pop()
        assert popped is self._sem_poison
        self.nc.clear_and_free_semaphores(list(self.sems.allocated().values()))
        self.nc.all_engine_barrier()

    tile_mod.TileContext._drain_and_barrier = patched
    tile_mod.TileContext._drain_split_patch = True


def _split_waits(nc):
    """This container's walrus accepts at most ONE sem wait per instruction;
    hoist extra waits onto same-engine NoOps inserted just before."""
    import bass_rust

    cnt = 0
    for f in nc.m.functions:
        for bb in f.blocks:
            new = []
            changed = False
            for inst in bb.instructions:
                si = inst.sync_info
                waits = list(si.on_wait) if si is not None else []
                if len(waits) > 1:
                    changed = True
                    for wt in waits[:-1]:
                        cnt += 1
                        nop = bass_rust.InstNoOp(
                            name=f"I-wsplit-{cnt}", engine=inst.engine
                        )
                        nop.sync_info = bass_rust.SyncInfo(on_wait=[wt], on_update=[])
                        new.append(nop)
                    inst.sync_info = bass_rust.SyncInfo(
                        on_wait=[waits[-1]], on_update=list(si.on_update)
                    )
                new.append(inst)
            if changed:
                bb.instructions = new
    return cnt


def build_program(split_waits=True):
    import concourse.bass as bass
    import concourse.mybir as mybir
    from concourse.tile import TileContext
    from concourse.masks import make_identity

    _patch_drain()
    f32 = mybir.dt.float32
    bf16 = mybir.dt.bfloat16
    i32 = mybir.dt.int32
    Alu = mybir.AluOpType
    Act = mybir.ActivationFunctionType
    Ax = mybir.AxisListType

    nc = bass.Bass()
    shiftm = nc.declare_dram_parameter("shiftm", [128, 256], f32, isOutput=False)
    zlw = nc.declare_dram_parameter("zlw", [512 * JW, TSEG], bf16, isOutput=False)
    ydN = nc.declare_dram_parameter("ydN", [BLOC * NSEG * TSEG, C], f32, isOutput=False)
    zbk = nc.declare_dram_parameter("zbk", [BLOC, NSEG * TSEG], f32, isOutput=False)
    dcol_in = nc.declare_dram_parameter("dcol", [128, JW], f32, isOutput=False)
    out_tl = nc.declare_dram_parameter("out_tl", [128, JW], f32, isOutput=True)
    out_tb = nc.declare_dram_parameter("out_tb", [128, JW], f32, isOutput=True)
    out_S = nc.declare_dram_parameter("out_S", [128, 1], f32, isOutput=True)
    out_logg = nc.declare_dram_parameter("out_logg", [BLOC, NSEG], f32, isOutput=True)
    out_logden = nc.declare_dram_parameter("out_logden", [BLOC, NSEG], f32, isOutput=True)

    TP2 = TSEG + 2   # CURL free width per j: [bbnd, l_0 .. l_TSEG]
    TP1 = TSEG + 1

    with TileContext(nc) as tc:
        with (
            tc.tile_pool(name="pers", bufs=1) as pers,
            tc.tile_pool(name="psum", bufs=1, space="PSUM") as psum,
            tc.tile_pool(name="load", bufs=6) as loadp,
            tc.tile_pool(name="work", bufs=6) as workp,
            tc.tile_pool(name="cols", bufs=2) as colp,
        ):
            # ---------------- persistent state ----------------
            CURL = [pers.tile([128, JW, TP2], bf16, name=f"curl{p}", tag=f"curl{p}")
                    for p in range(2)]
            CURB = [pers.tile([128, JW, TP1], bf16, name=f"curb{p}", tag=f"curb{p}")
                    for p in range(2)]
            QN = [pers.tile([128, JW, TP1], bf16, name=f"qn{p}", tag=f"qn{p}")
                  for p in range(3)]
            UT = pers.tile([128, JW, TP1], bf16, name="ut", tag="ut")
            QBX = [pers.tile([128, TP1], bf16, name=f"qbx{w}", tag=f"qbx{w}")
                   for w in range(NW)]
            RG = [pers.tile([128, TSEG], bf16, name=f"rg{w}", tag=f"rg{w}")
                  for w in range(NW)]
            PREVL = [pers.tile([128, TSEG], bf16, name=f"prevl{p}", tag=f"prevl{p}")
                     for p in range(2)]
            PREVB = [pers.tile([128, TSEG], bf16, name=f"prevb{p}", tag=f"prevb{p}")
                     for p in range(2)]
            B0 = [pers.tile([32, TP1], f32, name=f"b0{p}", tag=f"b0{p}")
                  for p in range(2)]
            B0u = pers.tile([32, TP1], f32, name="b0u", tag="b0u")
            Z33 = pers.tile([32, TP1], f32, name="z33", tag="z33")
            DCOL = pers.tile([128, JW], f32, name="dcol", tag="dcol")
            IDENT = pers.tile([128, 128], f32, name="ident", tag="ident")
            SHI = pers.tile([128, 128], f32, name="shi", tag="shi")
            SHFF = pers.tile([128, 128], f32, name="shff", tag="shff")
            SHF = pers.tile([128, 128], bf16, name="shf", tag="shf")
            SDEN = pers.tile([128, NSEG * BLOC], f32, name="sden", tag="sden")
            ZBK = pers.tile([BLOC, NSEG * TSEG], f32, name="zbkt", tag="zbkt")
            LOGG = pers.tile([BLOC, NSEG], f32, name="logg", tag="logg")
            LOGDEN = pers.tile([BLOC, NSEG], f32, name="logden", tag="logden")
            TLST = pers.tile([128, JW], f32, name="tlst", tag="tlst")
            TBST = pers.tile([128, JW], f32, name="tbst", tag="tbst")
            Scol = pers.tile([128, 1], f32, name="scol", tag="scol")
            rcol = pers.tile([128, 1], f32, name="rcol", tag="rcol")
            ARGS = pers.tile([128, 2], f32, name="args", tag="args")
            RCE = pers.tile([128, 2], f32, name="rce", tag="rce")

            # ---------------- init ----------------
            with nc.named_scope("init"):
                nc.sync.dma_start(out=DCOL[:], in_=dcol_in[:])
                nc.sync.dma_start(out=ZBK[:], in_=zbk[:])
                nc.sync.dma_start(out=SHI[:], in_=shiftm[:, 0:128])
                nc.sync.dma_start(out=SHFF[:], in_=shiftm[:, 128:256])
                nc.vector.tensor_copy(out=SHF[:], in_=SHFF[:])
                make_identity(nc, IDENT[:])
                for p in range(3):
                    nc.gpsimd.memset(QN[p][:], 0.0)
                    nc.gpsimd.memset(QN[p][:, :, 0:1], 1.0)
                for w in range(NW):
                    nc.gpsimd.memset(QBX[w][:], 0.0)
                    nc.gpsimd.memset(QBX[w][:, 0:1], 1.0)
                    nc.gpsimd.memset(RG[w][:], 0.0)
                nc.gpsimd.memset(CURL[1][:], 0.0)
                nc.gpsimd.memset(CURB[1][:], 0.0)
                nc.gpsimd.memset(PREVL[0][:], 0.0)
                nc.gpsimd.memset(PREVB[0][:], 0.0)
                nc.gpsimd.memset(B0[1][:], 0.0)
                nc.gpsimd.memset(B0[1][:, TSEG:TP1], 1.0)
                nc.gpsimd.memset(Z33[:], 0.0)
                nc.gpsimd.memset(Scol[:], 0.0)
                nc.gpsimd.memset(rcol[:], 1.0)

            # ---------------- per-segment stats ----------------
            ydN_v = ydN.rearrange("(b k t) c -> b k t c", b=BLOC, k=NSEG)

            def stats(k):
                with nc.named_scope(f"stats{k}"):
                    pbt = workp.tile([BLOC, TSEG], f32, name="pbt", tag="pbt")
                    gk = workp.tile([BLOC, TSEG], f32, name="gk", tag="gk")
                    scr = workp.tile([BLOC, TSEG], f32, name="scr", tag="scr")
                    scr2 = workp.tile([BLOC, TSEG], f32, name="scr2", tag="scr2")
                    rgkh = workp.tile([BLOC, TSEG], bf16, name="rgkh", tag="rgkh")
                    qbkh = workp.tile([BLOC, TSEG], bf16, name="qbkh", tag="qbkh")
                    denp = psum.tile([BLOC, TSEG], f32, name="denp", tag="denp")
                    nc.scalar.activation(out=pbt[:], in_=ZBK[:, k * TSEG:(k + 1) * TSEG],
                                         func=Act.Exp)
                    for b in range(BLOC):
                        zt = loadp.tile([TSEG, C], f32, name="zt", tag="zt")
                        pt = loadp.tile([TSEG, C], f32, name="pt", tag="pt")
                        nc.sync.dma_start(out=zt[:], in_=ydN_v[b, k, :, :])
                        nc.scalar.activation(out=pt[:], in_=zt[:], func=Act.Exp,
                                             accum_out=SDEN[:, 32 * k + b:32 * k + b + 1])
                    nc.tensor.transpose(out=denp[:], in_=SDEN[:, 32 * k:32 * k + 32],
                                        identity=IDENT[:])
                    # g = den/C + pb ; ln g (accum -> LOGG); rg = exp(-ln g);
                    # qb = pb * rg ; ln den (accum -> LOGDEN)
                    nc.vector.scalar_tensor_tensor(
                        out=gk[:], in0=denp[:], scalar=1.0 / C, in1=pbt[:],
                        op0=Alu.mult, op1=Alu.add)
                    nc.scalar.activation(out=scr[:], in_=gk[:], func=Act.Ln,
                                         accum_out=LOGG[:, k:k + 1])
                    nc.scalar.activation(out=rgkh[:], in_=scr[:], func=Act.Exp,
                                         scale=-1.0)
                    nc.scalar.activation(out=scr2[:], in_=denp[:], func=Act.Ln,
                                         accum_out=LOGDEN[:, k:k + 1])
                    nc.gpsimd.tensor_tensor(out=qbkh[:], in0=pbt[:], in1=rgkh[:],
                                            op=Alu.mult)
                    for q in range(NCH):
                        w = q + k
                        if w >= NW:
                            continue
                        rows = slice(32 * q, 32 * q + 32)
                        nc.sync.dma_start(out=QBX[w][rows, 1:TP1], in_=qbkh[:])
                        nc.sync.dma_start(out=RG[w][rows, :], in_=rgkh[:])

            # ---------------- per-wave emit ----------------
            zlw_base = [0] * NW
            acc = 0
            for w in range(NW):
                zlw_base[w] = acc
                qlo, qhi, p0, p1 = _wave_ranges(w)
                acc += (qhi - qlo + 1) * 32 * JW
            zlw_v = zlw.rearrange("(r j) t -> r j t", j=JW)

            def emit(w):
                P = w % 3
                qlo, qhi, p0, p1 = _wave_ranges(w)
                with nc.named_scope(f"emit{w}"):
                    for a, b in qsplit(p0, p1):
                        r0 = zlw_base[w] // JW + (a - p0)
                        nc.sync.dma_start(
                            out=QN[P][a:b, :, 1:TP1],
                            in_=zlw_v[r0:r0 + (b - a), :, :],
                        )
                        nc.scalar.activation(out=QN[P][a:b, :, 1:TP1],
                                             in_=QN[P][a:b, :, 1:TP1], func=Act.Exp)
                        nc.gpsimd.tensor_tensor(
                            out=QN[P][a:b, :, 1:TP1],
                            in0=QN[P][a:b, :, 1:TP1],
                            in1=RG[w][a:b, None, :].to_broadcast([b - a, JW, TSEG]),
                            op=Alu.mult)

            def b0_pre(w):
                # unrescaled b0 decay for wave w (issued under jloop(w-1)):
                # B0u[t] = B0o[TSEG] * prod qb; scaled by rcol at wstart(w).
                B0o = B0[1 - (w % 2)]
                nc.vector.tensor_tensor_scan(
                    out=B0u[:, 0:TP1], data0=Z33[:, 0:TP1],
                    data1=QBX[w][0:32, 0:TP1], initial=B0o[:, TSEG:TP1],
                    op0=Alu.add, op1=Alu.mult)

            # ---------------- waves ----------------
            stats(0)
            emit(0)
            b0_pre(0)

            for w in range(NW):
                P2 = w % 2
                P3 = w % 3
                qlo, qhi, p0, p1 = _wave_ranges(w)
                CL, CB = CURL[P2], CURB[P2]
                OL, OB = CURL[1 - P2], CURB[1 - P2]
                PL, PB = PREVL[P2], PREVB[P2]
                B0c, B0o = B0[P2], B0[1 - P2]
                r1 = 32 * (min(NCH - 1, w - 1) + 1)

                with nc.named_scope(f"wstart{w}"):
                    if w >= 1:
                        # PE: shifted row-31 tiles + shifted Scol (old Scol)
                        psh = psum.tile([128, 1], f32, name="psh", tag="psh")
                        pshm = psum.tile([128, 1], f32, name="pshm", tag="pshm")
                        ppl = psum.tile([128, TSEG], f32, name="ppl", tag="ppl")
                        ppb = psum.tile([128, TSEG], f32, name="ppb", tag="ppb")
                        nc.tensor.matmul(out=psh[:], lhsT=SHI[:], rhs=Scol[:],
                                         start=True, stop=True)
                        nc.tensor.matmul(out=ppl[:], lhsT=SHF[:],
                                         rhs=OL[:, JW - 1, 1:TP1], start=True,
                                         stop=True)
                        nc.tensor.matmul(out=ppb[:], lhsT=SHF[:],
                                         rhs=OB[:, JW - 1, 0:TSEG], start=True,
                                         stop=True)
                        m31 = colp.tile([128, 1], f32, name="m31", tag="m31")
                        mt = colp.tile([128, 1], f32, name="mt", tag="mt")
                        m_own = colp.tile([128, 1], f32, name="m_own", tag="m_own")
                        m_in = colp.tile([128, 1], f32, name="m_in", tag="m_in")
                        lg1 = colp.tile([128, 1], f32, name="lg1", tag="lg1")
                        lg2 = colp.tile([128, 1], f32, name="lg2", tag="lg2")
                        peak = colp.tile([128, 1], f32, name="peak", tag="peak")
                        snew = colp.tile([128, 1], f32, name="snew", tag="snew")
                        iw1 = colp.tile([128, 1], i32, name="iw1", tag="iw1")
                        iw2 = colp.tile([128, 1], i32, name="iw2", tag="iw2")
                        # incoming peak: max over row 31 of prev wave, shifted
                        nc.vector.tensor_reduce(out=m31[:], in_=OL[:, JW - 1, 1:TP2],
                                                axis=Ax.XY, op=Alu.max)
                        nc.vector.tensor_reduce(out=mt[:], in_=OB[:, JW - 1, 0:TP1],
                                                axis=Ax.XY, op=Alu.max)
                        nc.vector.tensor_tensor(out=m31[:], in0=m31[:], in1=mt[:],
                                                op=Alu.max)
                        nc.tensor.matmul(out=pshm[:], lhsT=SHFF[:], rhs=m31[:],
                                         start=True, stop=True)
                        # own peak: final column over all j
                        nc.vector.tensor_reduce(out=m_own[:],
                                                in_=OL[:, :, TP1:TP2],
                                                axis=Ax.XY, op=Alu.max)
                        nc.vector.tensor_reduce(out=lg1[:],
                                                in_=OB[:, :, TSEG:TP1],
                                                axis=Ax.XY, op=Alu.max)
                        nc.vector.tensor_tensor(out=m_own[:], in0=m_own[:],
                                                in1=lg1[:], op=Alu.max)
                        nc.vector.tensor_scalar(out=m_own[:], in0=m_own[:],
                                                scalar1=TINY, scalar2=1e38,
                                                op0=Alu.max, op1=Alu.min)
                        nc.vector.tensor_copy(out=m_in[:], in_=pshm[:])
                        if qlo == 0:
                            nc.vector.tensor_copy(out=m_in[0:32, :],
                                                  in_=B0o[:, TSEG:TP1])
                        nc.vector.tensor_scalar(out=m_in[:], in0=m_in[:],
                                                scalar1=TINY, scalar2=1e38,
                                                op0=Alu.max, op1=Alu.min)
                        # lg = ln(m) via exponent-bits approximation
                        nc.vector.tensor_copy(out=iw1[:],
                                              in_=m_own[:].bitcast(i32))
                        nc.vector.tensor_copy(out=lg1[:], in_=iw1[:])
                        nc.vector.tensor_scalar(out=lg1[:], in0=lg1[:],
                                                scalar1=KLN, scalar2=-FB * KLN,
                                                op0=Alu.mult, op1=Alu.add)
                        nc.vector.tensor_copy(out=iw2[:],
                                              in_=m_in[:].bitcast(i32))
                        nc.vector.tensor_copy(out=lg2[:], in_=iw2[:])
                        nc.vector.tensor_scalar(out=lg2[:], in0=lg2[:],
                                                scalar1=KLN, scalar2=-FB * KLN,
                                                op0=Alu.mult, op1=Alu.add)
                        nc.vector.tensor_tensor(out=lg1[:], in0=lg1[:],
                                                in1=Scol[:], op=Alu.subtract)
                        nc.vector.tensor_tensor(out=lg2[:], in0=lg2[:],
                                                in1=psh[:], op=Alu.subtract)
                        nc.vector.tensor_tensor(out=peak[:], in0=lg1[:],
                                                in1=lg2[:], op=Alu.max)
                        nc.vector.tensor_scalar(out=snew[:], in0=peak[:],
                                                scalar1=-1.0, scalar2=LT,
                                                op0=Alu.mult, op1=Alu.add)
                        # rcol = exp(snew - Scol_old); corr = exp(Scol_new - Sshift)
                        nc.vector.tensor_tensor(out=ARGS[:, 0:1], in0=snew[:],
                                                in1=Scol[:], op=Alu.subtract)
                        for a, b in qsplit(p0, r1):
                            nc.vector.tensor_copy(out=Scol[a:b, :], in_=snew[a:b, :])
                        nc.vector.tensor_tensor(out=ARGS[:, 1:2], in0=Scol[:],
                                                in1=psh[:], op=Alu.subtract)
                        nc.scalar.activation(out=RCE[:], in_=ARGS[:], func=Act.Exp)
                        nc.vector.tensor_copy(out=rcol[:], in_=RCE[:, 0:1])
                        # prev-row handoff at incoming scale correction
                        nc.vector.tensor_scalar_mul(PL[:], ppl[:], RCE[:, 1:2])
                        nc.vector.tensor_scalar_mul(PB[:], ppb[:], RCE[:, 1:2])
                    if w <= NSEG - 1:
                        # chunk-0 row: b0 decay, rescaled
                        nc.vector.tensor_scalar_mul(B0c[:, 0:TP1], B0u[:, 0:TP1],
                                                    rcol[0:32, :])
                        nc.vector.tensor_scalar_mul(PB[0:32, :], B0u[:, 0:TSEG],
                                                    rcol[0:32, :])
                    # boundary handoff into this wave's tiles
                    nc.gpsimd.tensor_scalar_mul(UT[:, :, 0:1], OL[:, :, TP1:TP2],
                                                rcol[:])
                    nc.gpsimd.tensor_scalar_mul(CL[:, :, 0:1], OB[:, :, TSEG:TP1],
                                                rcol[:])

                with nc.named_scope(f"jloop{w}"):
                    for j in range(JW):
                        if j == 0:
                            plp, pbp = PL[:, :], PB[:, :]
                        else:
                            plp = CL[:, j - 1, 1:TP1]
                            pbp = CB[:, j - 1, 0:TSEG]
                        nc.vector.scalar_tensor_tensor(
                            out=UT[:, j, 1:TP1], in0=plp,
                            scalar=DCOL[:, j:j + 1], in1=pbp,
                            op0=Alu.mult, op1=Alu.add)
                        nc.vector.tensor_tensor_scan(
                            out=CL[:, j, 1:TP2], data0=UT[:, j, 0:TP1],
                            data1=QN[P3][:, j, 0:TP1], initial=0.0,
                            op0=Alu.add, op1=Alu.mult)
                        nc.vector.tensor_tensor_scan(
                            out=CB[:, j, 0:TP1], data0=CL[:, j, 0:TP1],
                            data1=QBX[w][:, 0:TP1], initial=0.0,
                            op0=Alu.add, op1=Alu.mult)

                # next wave's stats/emit/b0 run under this wave's jloop
                if w + 1 < NW:
                    if w + 1 <= NSEG - 1:
                        stats(w + 1)
                    emit(w + 1)
                    if w + 1 <= NSEG - 1:
                        b0_pre(w + 1)

                if w >= NSEG - 1:  # chunk q = w - (NSEG-1) finished segment 3
                    q = w - (NSEG - 1)
                    rows = slice(32 * q, 32 * q + 32)
                    nc.gpsimd.tensor_copy(out=TLST[rows, :],
                                          in_=CL[rows, :, TP1:TP2])
                    nc.gpsimd.tensor_copy(out=TBST[rows, :],
                                          in_=CB[rows, :, TSEG:TP1])

            nc.sync.dma_start(out=out_tl[:], in_=TLST[:])
            nc.sync.dma_start(out=out_tb[:], in_=TBST[:])
            nc.sync.dma_start(out=out_S[:], in_=Scol[:])
            nc.sync.dma_start(out=out_logg[:], in_=LOGG[:])
            nc.sync.dma_start(out=out_logden[:], in_=LOGDEN[:])
    if split_waits:
        _split_waits(nc)
    return nc


def host_prep(y_true, y_pred):
    """Build per-core input maps."""
    import ml_dtypes

    y_true = np.asarray(y_true).astype(np.int32)
    y_pred = np.asarray(y_pred).astype(np.float32)
    shiftm = np.zeros((128, 256), np.float32)
    for m in range(128):
        shiftm[m - 32 if m >= 32 else m, m] = 1.0          # SHI
        if m >= 32:
            shiftm[m - 32, 128 + m] = 1.0                  # SHF
    in_maps = []
    for core in range(NCORE):
        yt = y_true[core * BLOC:(core + 1) * BLOC]        # [32, 128]
        yp = y_pred[core * BLOC:(core + 1) * BLOC]        # [32, 512, 128]
        ydN = np.ascontiguousarray(yp).reshape(BLOC * NSEG * TSEG, C)
        zbk = np.ascontiguousarray(yp[:, :, BLANK])       # [32, 512]
        # gathered label logits, wave-block layout
        zg = np.take_along_axis(yp, yt.astype(np.int64)[:, None, :], axis=2)
        zgt = zg.transpose(0, 2, 1)                       # [32 b, 128 i, 512 t]
        zlw = np.zeros((512 * JW, TSEG), np.float32)
        base = 0
        for w in range(NW):
            qlo, qhi, p0, p1 = _wave_ranges(w)
            for q in range(qlo, qhi + 1):
                k = w - q
                blk = zgt[:, 32 * q:32 * q + JW, k * TSEG:(k + 1) * TSEG]
                n = BLOC * JW
                zlw[base:base + n] = blk.reshape(n, TSEG)
                base += n
        zlw = zlw.astype(ml_dtypes.bfloat16)
        dcol = np.zeros((128, JW), np.float32)
        for q in range(NCH):
            for j in range(JW):
                i = 32 * q + j + 1
                if i >= 2:
                    dcol[32 * q:32 * q + 32, j] = (
                        yt[:, i - 1] != yt[:, i - 2]).astype(np.float32)
        in_maps.append({"zlw": zlw, "ydN": ydN, "zbk": zbk, "dcol": dcol,
                        "shiftm": shiftm})
    return in_maps


def host_finish(y_true, results):
    y_true = np.asarray(y_true)
    ll = (y_true != 0).sum(axis=1).astype(np.int64)        # [256]
    losses = np.zeros(B, np.float64)
    for core in range(NCORE):
        res = results[core]
        tl = np.asarray(res["out_tl"], dtype=np.float64)
        tb = np.asarray(res["out_tb"], dtype=np.float64)
        S, logg, logden = res["out_S"], res["out_logg"], res["out_logden"]
        for b in range(BLOC):
            gb = core * BLOC + b
            l = int(ll[gb])
            q, j = (l - 1) // 32, (l - 1) % 32
            p = 32 * q + b
            logP = np.log(tl[p, j] + tb[p, j])
            losses[gb] = -(logP + logg[b].sum() - logden[b].sum() - S[p, 0])
    return np.float32(losses.mean())


def _get_program():
    if "nc" not in _PROG:
        _PROG["nc"] = build_program()
    return _PROG["nc"]


def kernel(y_true: np.ndarray, y_pred: np.ndarray) -> np.ndarray:
    from concourse.bass_utils import run_bass_kernel_spmd

    nc = _get_program()
    in_maps = host_prep(y_true, y_pred)
    res = run_bass_kernel_spmd(nc, in_maps, core_ids=list(range(NCORE)))
    return host_finish(y_true, res.results)
